# revision 1
# baseline (speedup 1.0000x reference)
"""Trainium2 Bass kernel for nn_AttentionModel (decomposable attention).

Strategy (8 NeuronCores, SPMD, no collectives needed):
  Every core receives the full sen1/sen2 (transposed + ones-augmented forms,
  host-prepped) and computes full F projections FbT/FaT = mlp_F(sen)^T in
  feature-major layout (redundantly; it's cheap). The E = Fa @ Fb^T matrix is
  never materialized in HBM: each core makes two fused passes over its block
  of E, computed on the fly in PSUM, exp'd on the Scalar engine:
    - beta pass: E^T-oriented tiles [j-part, i-free] for the core's sen1 row
      shard; numerator matmuls contract j against sen2 (ones column appended
      gives the softmax denominator for free in PSUM column 300).
    - alpha pass: E-oriented tiles [i-part, j-free] for the core's sen2 row
      shard over ALL i; contracting i against sen1 gives the column-softmax
      numerator/denominator with no cross-core reduction at all.
  (exp needs no max subtraction: E is a product of relu'd activations and
  lives in [0.8, 10.2]; exp is safely in fp32 range.)
  beta/alpha are divided, transposed (PE transpose) into feature-major fp32r
  and fed to the G MLP; per-token sums come from the Scalar engine's
  accum_out. Each core emits a [4,100] partial v-sum; the host sums the 8
  partials and applies the tiny H classifier + softmax in numpy.

  FbT-full and FaT-full never coexist (beta needs FbT-full+FaT-shard, alpha
  needs FaT-full+FbT-shard), so they share SBUF slots via tag reuse, split
  into 1024-column tiles for fine-grained cross-phase pipelining.

  Heavy matmuls run as float32r (11-bit-mantissa fp32, 4x the fp32 PE
  rate); the softmax-numerator matmuls run in bf16 (measured ~50us/call
  faster via fast-weight-load on their per-tile lhsT reloads). The model
  output is a saturated 3-way softmax and tolerates far more noise than
  these formats introduce (verified against fp64; rel err 3.6e-3).
"""
import sys
sys.path.insert(0, "/opt/trn_rl_repo")

import numpy as np
import concourse.bass as bass
import concourse.mybir as mybir
from concourse import tile
from concourse.vector_clock import ScopedClock
from concourse.bass_utils import run_bass_kernel_spmd

FP32R = mybir.dt.float32r
F32 = mybir.dt.float32
AF = mybir.ActivationFunctionType
BF16 = mybir.dt.bfloat16

# ---------------------------------------------------------------------------
# walrus-compat TileContext: the installed walrus rejects >1 sync wait per
# instruction; split extra waits onto same-engine NoOps.
# ---------------------------------------------------------------------------
_noop_ctr = [0]


def _split_multi_waits(nc, max_waits=1):
    for fn in nc.m.functions:
        for bb in fn.blocks:
            out = []
            for inst in bb.instructions:
                si = getattr(inst, "sync_info", None)
                waits = list(si.on_wait) if (si and si.on_wait) else []
                if len(waits) > max_waits:
                    keep, rest = waits[:max_waits], waits[max_waits:]
                    for i in range(0, len(rest), max_waits):
                        _noop_ctr[0] += 1
                        nop = mybir.InstNoOp(
                            name=f"I-splitw-{_noop_ctr[0]}", ins=[], outs=[]
                        )
                        nop.engine = inst.engine
                        nop.sync_info = mybir.SyncInfo(
                            on_wait=rest[i : i + max_waits], on_update=[]
                        )
                        out.append(nop)
                    inst.sync_info = mybir.SyncInfo(
                        on_wait=keep, on_update=list(si.on_update or [])
                    )
                out.append(inst)
            bb.instructions[:] = out


class _TC(tile.TileContext):
    def _drain_and_barrier(self, tick_clock, wait_clock):
        probe = self.nc.sync.nop()
        wait_clock.add_sem_waits(
            probe.ins, ScopedClock({None: tick_clock.global_clock})
        )
        waits = list(probe.ins.sync_info.on_wait or []) if probe.ins.sync_info else []
        probe.ins.sync_info = mybir.SyncInfo(on_wait=waits[:1], on_update=[])
        for i in range(1, len(waits)):
            nxt = self.nc.sync.nop()
            nxt.ins.sync_info = mybir.SyncInfo(on_wait=waits[i : i + 1], on_update=[])
        self.nc.sync.drain()
        self.nc.all_engine_barrier()
        assert self.sems is not None
        popped = self.nc._tile_sem_poison_stack.pop()
        assert popped is self._sem_poison
        self.nc.clear_and_free_semaphores(list(self.sems.allocated().values()))
        self.nc.all_engine_barrier()

    def __exit__(self, exc_type, exc_val, exc_tb):
        r = super().__exit__(exc_type, exc_val, exc_tb)
        if exc_type is None:
            _split_multi_waits(self.nc)
        return r


# ---------------------------------------------------------------------------
# problem constants (hardcoded per the harness contract)
# ---------------------------------------------------------------------------
L = 8192          # tokens per sentence
EMB = 300
FD = 200          # F/G output dim
CORES = 8
SH = L // CORES   # per-core shard (1024)
SO = 304          # ones-augmented width (300 sen + 1 ones + 3 pad; even for fp32r)

_nc_cache = {}


def _chunks(total, step):
    out = []
    o = 0
    while o < total:
        w = min(step, total - o)
        out.append((o, w))
        o += w
    return out


def _build(L=L, SH=SH, reps=1):
    """Build the SPMD per-core Bass program."""
    nc = bass.Bass()
    JBN = L // 128           # 128-row blocks over the full sequence
    NHALF = _chunks(SH, 512)  # i (or j) halves of the shard for PSUM budget

    # ---- I/O ----
    s1t = nc.dram_tensor("s1t", [EMB, L], FP32R, kind="ExternalInput")
    s2t = nc.dram_tensor("s2t", [EMB, L], FP32R, kind="ExternalInput")
    s1ts = nc.dram_tensor("s1ts", [EMB, SH], FP32R, kind="ExternalInput")
    s2ts = nc.dram_tensor("s2ts", [EMB, SH], FP32R, kind="ExternalInput")
    s1o = nc.dram_tensor("s1o", [L, SO], BF16, kind="ExternalInput")
    s2o = nc.dram_tensor("s2o", [L, SO], BF16, kind="ExternalInput")
    fw1t = nc.dram_tensor("fw1t", [EMB, FD], FP32R, kind="ExternalInput")
    fw2t = nc.dram_tensor("fw2t", [FD, FD], FP32R, kind="ExternalInput")
    gw1t = nc.dram_tensor("gw1t", [2 * EMB, FD], FP32R, kind="ExternalInput")
    gw2t = nc.dram_tensor("gw2t", [FD, FD], FP32R, kind="ExternalInput")
    gw1l = nc.dram_tensor("gw1l", [2 * EMB, FD], FP32R, kind="ExternalInput")
    gw2l = nc.dram_tensor("gw2l", [FD, FD], FP32R, kind="ExternalInput")
    fb1 = nc.dram_tensor("fb1", [FD, 1], F32, kind="ExternalInput")
    fb2 = nc.dram_tensor("fb2", [FD, 1], F32, kind="ExternalInput")
    gb1 = nc.dram_tensor("gb1", [FD, 1], F32, kind="ExternalInput")
    gb2 = nc.dram_tensor("gb2", [FD, 1], F32, kind="ExternalInput")
    ident = nc.dram_tensor("ident", [128, 128], F32, kind="ExternalInput")
    vsum = nc.dram_tensor("vsum", [4, 100], F32, kind="ExternalOutput")

    with _TC(nc) as tc:
        with (
            tc.tile_pool(name="persist", bufs=1) as pp,
            tc.tile_pool(name="work", bufs=2) as wp,
        ):
            # ---- constants ----
            fw1_sb = [pp.tile([100, FD], FP32R, tag=f"fw1_{i}", name=f"fw1_{i}") for i in range(3)]
            fw2_sb = [pp.tile([100, FD], FP32R, tag=f"fw2_{i}", name=f"fw2_{i}") for i in range(2)]
            gw1_sb = [pp.tile([100, FD], FP32R, tag=f"gw1_{i}", name=f"gw1_{i}") for i in range(6)]
            gw2_sb = [pp.tile([100, FD], FP32R, tag=f"gw2_{i}", name=f"gw2_{i}") for i in range(2)]
            gw1l_sb = [pp.tile([100, FD], FP32R, tag=f"gw1l_{i}", name=f"gw1l_{i}") for i in range(6)]
            gw2l_sb = [pp.tile([100, FD], FP32R, tag=f"gw2l_{i}", name=f"gw2l_{i}") for i in range(2)]
            for i, t in enumerate(fw1_sb):
                nc.sync.dma_start(t[:], fw1t[i * 100:(i + 1) * 100, :])
            for i, t in enumerate(fw2_sb):
                nc.sync.dma_start(t[:], fw2t[i * 100:(i + 1) * 100, :])
            biases = {}
            for nm, dr in (("fb1", fb1), ("fb2", fb2), ("gb1", gb1), ("gb2", gb2)):
                for h in range(2):
                    t = pp.tile([100, 1], F32, tag=f"{nm}_{h}", name=f"{nm}_{h}")
                    nc.sync.dma_start(t[:], dr[h * 100:(h + 1) * 100, :])
                    biases[(nm, h)] = t
            id_sb = pp.tile([128, 128], F32, tag="ident", name="id_sb")

            # ---- persistent shard projections ----
            FaTs = [pp.tile([100, SH], FP32R, tag=f"FaTs{h}", name=f"FaTs{h}") for h in range(2)]
            FbTs = [pp.tile([100, SH], FP32R, tag=f"FbTs{h}", name=f"FbTs{h}") for h in range(2)]

            FW = 1024
            NF = L // FW

            def alloc_Ffull(nm):
                return [[pp.tile([100, FW], FP32R, tag=f"Ffull{h}_{q}",
                                 name=f"{nm}{h}_{q}") for q in range(NF)]
                        for h in range(2)]

            def alloc_abT(nm):
                return [pp.tile([100, SH], FP32R, tag=f"{nm}{f}", name=f"{nm}{f}")
                        for f in range(3)]

            # ---------------- F MLP (feature-major) ----------------
            def f_mlp(ps, src_dram, width, out0, out1):
                for (off, w) in _chunks(width, 512):
                    xc = [wp.tile([100, 512], FP32R, tag=f"f_x{c}", name=f"f_x{c}")
                          for c in range(3)]
                    for c in range(3):
                        nc.sync.dma_start(
                            xc[c][:, :w],
                            src_dram[c * 100:(c + 1) * 100, off:off + w],
                        )
                    h1 = []
                    for m in range(2):
                        hp = ps.tile([100, 512], F32, tag=f"f_h1{m}", name=f"f_h1{m}")
                        for c in range(3):
                            nc.tensor.matmul(
                                hp[:, :w], fw1_sb[c][:, m * 100:(m + 1) * 100],
                                xc[c][:, :w], start=(c == 0), stop=(c == 2),
                            )
                        hs = wp.tile([100, 512], FP32R, tag=f"f_h1s{m}", name=f"f_h1s{m}")
                        nc.vector.tensor_scalar(
                            hs[:, :w], hp[:, :w], biases[("fb1", m)][:], 0.0,
                            mybir.AluOpType.add, mybir.AluOpType.max,
                        )
                        h1.append(hs)
                    for m in range(2):
                        hp = ps.tile([100, 512], F32, tag=f"f_h2{m}", name=f"f_h2{m}")
                        for c in range(2):
                            nc.tensor.matmul(
                                hp[:, :w], fw2_sb[c][:, m * 100:(m + 1) * 100],
                                h1[c][:, :w], start=(c == 0), stop=(c == 1),
                            )
                        dst = (out0 if m == 0 else out1)
                        nc.vector.tensor_scalar(
                            dst(off, w), hp[:, :w], biases[("fb2", m)][:], 0.0,
                            mybir.AluOpType.add, mybir.AluOpType.max,
                        )

            # ------- fused E pass + softmax numerators -------
            def softmax_pass(lhs_full, rhs_shard, so_dram, outT):
                with tc.tile_pool(name="sp_sb", bufs=2) as sp, \
                     tc.tile_pool(name="sp_ps", bufs=1, space="PSUM") as ps:
                    assert JBN % 2 == 0
                    for (hoff, hw) in NHALF:
                        nsl = (hw + 127) // 128
                        accs = [ps.tile([128, SO], F32, tag=f"acc{k}", name=f"acc{k}")
                                for k in range(nsl)]
                        for jp in range(JBN // 2):
                            so_ts = []
                            for u in range(2):
                                jb = jp * 2 + u
                                so_t = sp.tile([128, SO], BF16, tag=f"so{u}", bufs=3,
                                               name=f"so_t{u}")
                                nc.sync.dma_start(
                                    so_t[:], so_dram[jb * 128:(jb + 1) * 128, :]
                                )
                                so_ts.append(so_t)
                            e_ps = ps.tile([128, 1024], F32, tag="e", bufs=2,
                                           name="e_ps")
                            for u in range(2):
                                jb = jp * 2 + u
                                for c in range(2):
                                    q, qo = (jb * 128) // 1024, (jb * 128) % 1024
                                    nc.tensor.matmul(
                                        e_ps[:, u * hw:u * hw + hw],
                                        lhs_full[c][q][:, qo:qo + 128],
                                        rhs_shard[c][:, hoff:hoff + hw],
                                        start=(c == 0), stop=(c == 1),
                                    )
                            p_t = sp.tile([128, 1024], BF16, tag="p", bufs=3, name="p_t")
                            nc.scalar.activation(
                                p_t[:, :2 * hw], e_ps[:, :2 * hw], AF.Exp
                            )
                            for u in range(2):
                                for k in range(nsl):
                                    kw = min(128, hw - k * 128)
                                    nc.tensor.matmul(
                                        accs[k][:kw, :],
                                        p_t[:, u * hw + k * 128:u * hw + k * 128 + kw],
                                        so_ts[u][:],
                                        start=(jp == 0 and u == 0),
                                        stop=(jp == JBN // 2 - 1 and u == 1),
                                    )
                        for k in range(nsl):
                            kw = min(128, hw - k * 128)
                            rec = sp.tile([128, 1], F32, tag="rec", bufs=2, name="rec")
                            nc.vector.reciprocal(rec[:kw, :], accs[k][:kw, 300:301])
                            dv = sp.tile([128, 300], F32, tag="dv", bufs=2, name="dv")
                            nc.vector.tensor_scalar_mul(
                                dv[:kw, :], accs[k][:kw, 0:300], rec[:kw, :]
                            )
                            for f in range(3):
                                trp = ps.tile([100, 128], F32, tag="e", bufs=2, name="trp")
                                nc.tensor.transpose(
                                    trp[:, :kw], dv[:kw, f * 100:(f + 1) * 100],
                                    id_sb[:],
                                )
                                nc.vector.tensor_copy(
                                    outT[f][:, hoff + k * 128:hoff + k * 128 + kw],
                                    trp[:, :kw],
                                )

            # -------- G MLP + token sums --------
            def g_pass(x_dram, xT, row0):
                with tc.tile_pool(name="g_sb", bufs=2) as gp, \
                     tc.tile_pool(name="g_ps", bufs=2, space="PSUM") as ps:
                    nblk = _chunks(SH, 512)
                    vacc = [gp.tile([100, len(nblk)], F32, tag=f"vacc{m}", bufs=1,
                                    name=f"vacc{m}")
                            for m in range(2)]
                    for bi, (off, w) in enumerate(nblk):
                        xc = [gp.tile([100, 512], FP32R, tag=f"g_x{c}", name=f"g_x{c}")
                              for c in range(3)]
                        for c in range(3):
                            nc.sync.dma_start(
                                xc[c][:, :w],
                                x_dram[c * 100:(c + 1) * 100, off:off + w],
                            )
                        rhs6 = [t[:, :w] for t in xc] + \
                               [t[:, off:off + w] for t in xT]
                        h1 = []
                        for m in range(2):
                            hp = ps.tile([100, 512], F32, tag=f"g_h1{m}", name=f"g_h1{m}")
                            for c in range(12):
                                wsb = (gw1_sb if c < 6 else gw1l_sb)[c % 6]
                                nc.tensor.matmul(
                                    hp[:, :w], wsb[:, m * 100:(m + 1) * 100],
                                    rhs6[c % 6], start=(c == 0), stop=(c == 11),
                                )
                            hs = gp.tile([100, 512], FP32R, tag=f"g_h1s{m}", name=f"g_h1s{m}")
                            nc.vector.tensor_scalar(
                                hs[:, :w], hp[:, :w], biases[("gb1", m)][:], 0.0,
                                mybir.AluOpType.add, mybir.AluOpType.max,
                            )
                            h1.append(hs)
                        for m in range(2):
                            hp = ps.tile([100, 512], F32, tag=f"g_h2{m}", name=f"g_h2{m}")
                            for c in range(4):
                                wsb = (gw2_sb if c < 2 else gw2l_sb)[c % 2]
                                nc.tensor.matmul(
                                    hp[:, :w], wsb[:, m * 100:(m + 1) * 100],
                                    h1[c % 2][:, :w], start=(c == 0), stop=(c == 3),
                                )
                            vv = gp.tile([100, 512], F32, tag=f"g_v{m}", name=f"g_v{m}")
                            nc.scalar.activation(
                                vv[:, :w], hp[:, :w], AF.Relu,
                                bias=biases[("gb2", m)][:],
                                accum_out=vacc[m][:, bi:bi + 1],
                            )
                    for m in range(2):
                        tot = gp.tile([100, 1], F32, tag=f"tot{m}", bufs=1, name=f"tot{m}")
                        if len(nblk) == 1:
                            nc.vector.tensor_copy(tot[:], vacc[m][:, 0:1])
                        else:
                            nc.vector.tensor_add(
                                tot[:], vacc[m][:, 0:1], vacc[m][:, 1:2]
                            )
                            for bi in range(2, len(nblk)):
                                nc.vector.tensor_add(
                                    tot[:], tot[:], vacc[m][:, bi:bi + 1]
                                )
                        nc.sync.dma_start(vsum[row0 + m:row0 + m + 1, :], tot[:])

            # ---------------- schedule ----------------
            def fdst(tiles):
                return lambda o, w: tiles[o // FW][:, o % FW:o % FW + w]

            for _rep in range(reps):
              FbT = alloc_Ffull("FbT")
              with tc.tile_pool(name="ps1a", bufs=2, space="PSUM") as ps1:
                  f_mlp(ps1, s1ts, SH, lambda o, w: FaTs[0][:, o:o + w],
                        lambda o, w: FaTs[1][:, o:o + w])
                  f_mlp(ps1, s2ts, SH, lambda o, w: FbTs[0][:, o:o + w],
                        lambda o, w: FbTs[1][:, o:o + w])
                  f_mlp(ps1, s2t, L, fdst(FbT[0]), fdst(FbT[1]))
              if _rep == 0:
                  for i, t in enumerate(gw1_sb):
                      nc.sync.dma_start(t[:], gw1t[i * 100:(i + 1) * 100, :])
                  for i, t in enumerate(gw2_sb):
                      nc.sync.dma_start(t[:], gw2t[i * 100:(i + 1) * 100, :])
                  for i, t in enumerate(gw1l_sb):
                      nc.sync.dma_start(t[:], gw1l[i * 100:(i + 1) * 100, :])
                  for i, t in enumerate(gw2l_sb):
                      nc.sync.dma_start(t[:], gw2l[i * 100:(i + 1) * 100, :])
                  nc.sync.dma_start(id_sb[:], ident[:])
              betaT = alloc_abT("betaT")
              softmax_pass(FbT, FaTs, s2o, betaT)

              FaT = alloc_Ffull("FaT")
              with tc.tile_pool(name="ps1b", bufs=2, space="PSUM") as ps1:
                  f_mlp(ps1, s1t, L, fdst(FaT[0]), fdst(FaT[1]))
              g_pass(s1ts, betaT, 0)
              alphaT = alloc_abT("alphaT")
              softmax_pass(FaT, FbTs, s1o, alphaT)
              g_pass(s2ts, alphaT, 2)

    return nc


def _round_fp32r(a):
    ai = np.ascontiguousarray(a, dtype=np.float32).view(np.uint32)
    return ((ai + np.uint32(1 << 11)) & np.uint32(0xFFFFF000)).view(np.float32)


def kernel(sen1, sen2, F_w1, F_b1, F_w2, F_b2,
           G_w1, G_b1, G_w2, G_b2, H_w1, H_b1, H_w2, H_b2):
    sen1 = np.asarray(sen1, np.float32)
    sen2 = np.asarray(sen2, np.float32)
    Lc, E = sen1.shape
    assert (Lc, E) == (L, EMB)

    key = ("nc", L, SH)
    if key not in _nc_cache:
        _nc_cache[key] = _build(L, SH)
    nc = _nc_cache[key]

    def onesaug(s):
        import ml_dtypes
        o = np.zeros((L, SO), np.float32)
        o[:, :EMB] = s
        o[:, EMB] = 1.0
        return o.astype(ml_dtypes.bfloat16)

    in_common = {
        "s1t": _round_fp32r(sen1.T),
        "s2t": _round_fp32r(sen2.T),
        "s1o": onesaug(sen1),
        "s2o": onesaug(sen2),
        "fw1t": _round_fp32r(np.asarray(F_w1, np.float32).T),
        "fw2t": _round_fp32r(np.asarray(F_w2, np.float32).T),
        "gw1t": _round_fp32r(np.asarray(G_w1, np.float32).T),
        "gw2t": _round_fp32r(np.asarray(G_w2, np.float32).T),
        "gw1l": _round_fp32r(np.asarray(G_w1, np.float32).T
                             - _round_fp32r(np.asarray(G_w1, np.float32).T)),
        "gw2l": _round_fp32r(np.asarray(G_w2, np.float32).T
                             - _round_fp32r(np.asarray(G_w2, np.float32).T)),
        "fb1": np.asarray(F_b1, np.float32).reshape(FD, 1),
        "fb2": np.asarray(F_b2, np.float32).reshape(FD, 1),
        "gb1": np.asarray(G_b1, np.float32).reshape(FD, 1),
        "gb2": np.asarray(G_b2, np.float32).reshape(FD, 1),
        "ident": np.eye(128, dtype=np.float32),
    }
    in_maps = []
    for c in range(CORES):
        m = dict(in_common)
        m["s1ts"] = np.ascontiguousarray(in_common["s1t"][:, c * SH:(c + 1) * SH])
        m["s2ts"] = np.ascontiguousarray(in_common["s2t"][:, c * SH:(c + 1) * SH])
        in_maps.append(m)

    res = run_bass_kernel_spmd(nc, in_maps, list(range(CORES))).results

    hx = np.zeros(2 * FD, np.float64)
    for c in range(CORES):
        v = res[c]["vsum"].astype(np.float64)
        hx[0:100] += v[0]
        hx[100:200] += v[1]
        hx[200:300] += v[2]
        hx[300:400] += v[3]

    h = np.maximum(hx @ np.asarray(H_w1, np.float64).T + np.asarray(H_b1, np.float64), 0)
    logits = h @ np.asarray(H_w2, np.float64).T + np.asarray(H_b2, np.float64)
    p = np.exp(logits - logits.max())
    p = p / p.sum()
    return p.astype(np.float32)



# revision 26
# speedup vs baseline: 1.0033x; 1.0033x over previous
"""Trainium2 Bass kernel for nn_AttentionModel (decomposable attention).

Strategy (8 NeuronCores, SPMD, sequence-parallel over sen1 rows of E):
  Each core owns a 1024-row shard of E = Fa @ Fb^T and computes it ONCE, in
  [j-part, i-free] orientation, exp'd on the Scalar engine into bf16 P tiles:
    - beta (row softmax): P-block matmuls against ones-augmented sen2 tiles
      accumulate [i, 304] numerator+denominator in 8 PSUM-resident
      accumulators across the whole j loop; fully core-local.
    - alpha (col softmax): P tiles are PE-transposed ([i-part, j]) and
      contracted against the core's ones-augmented sen1 shard, giving
      per-core partials N_c[j, 304] = sum_{i in shard} P_ij * [s1_i | 1].
      Partials stream to DRAM (bf16) and two pipelined ReduceScatters (first
      issued mid-loop over a row-permuted layout) deliver each core the
      summed [1024, 304] block for its sen2 shard; alpha = N[:, :300]/N[:, 300].
  F projections: Fa for the core's shard plus Fb for ALL of sen2 (redundant
  per-core compute is cheaper than gathering). F/E operands run bf16 (the
  jax reference is itself fp32; validated ~4e-3 end-to-end), G keeps the
  fp32r + fp32r-residual split weights (needed: plain fp32r G is 2.3e-2).
  exp needs no max subtraction: E lives in [0.8, 10.2] (relu'd activations).
  Per-token G sums come from the Scalar engine's accum_out; each core emits
  a [4,100] partial v-sum; the host sums the 8 partials and applies the tiny
  H classifier + softmax in numpy.
"""
import sys
sys.path.insert(0, "/opt/trn_rl_repo")

import numpy as np
import concourse.bass as bass
import concourse.mybir as mybir
from concourse import tile
from concourse.vector_clock import ScopedClock
from concourse.bass_utils import run_bass_kernel_spmd

FP32R = mybir.dt.float32r
F32 = mybir.dt.float32
AF = mybir.ActivationFunctionType
BF16 = mybir.dt.bfloat16

# ---------------------------------------------------------------------------
# walrus-compat TileContext: the installed walrus rejects >1 sync wait per
# instruction; split extra waits onto same-engine NoOps.
# ---------------------------------------------------------------------------
_noop_ctr = [0]


def _split_multi_waits(nc, max_waits=1):
    for fn in nc.m.functions:
        for bb in fn.blocks:
            out = []
            for inst in bb.instructions:
                si = getattr(inst, "sync_info", None)
                waits = list(si.on_wait) if (si and si.on_wait) else []
                if len(waits) > max_waits:
                    keep, rest = waits[:max_waits], waits[max_waits:]
                    for i in range(0, len(rest), max_waits):
                        _noop_ctr[0] += 1
                        nop = mybir.InstNoOp(
                            name=f"I-splitw-{_noop_ctr[0]}", ins=[], outs=[]
                        )
                        nop.engine = inst.engine
                        nop.sync_info = mybir.SyncInfo(
                            on_wait=rest[i : i + max_waits], on_update=[]
                        )
                        out.append(nop)
                    inst.sync_info = mybir.SyncInfo(
                        on_wait=keep, on_update=list(si.on_update or [])
                    )
                out.append(inst)
            bb.instructions[:] = out


class _TC(tile.TileContext):
    def _drain_and_barrier(self, tick_clock, wait_clock):
        probe = self.nc.sync.nop()
        wait_clock.add_sem_waits(
            probe.ins, ScopedClock({None: tick_clock.global_clock})
        )
        waits = list(probe.ins.sync_info.on_wait or []) if probe.ins.sync_info else []
        probe.ins.sync_info = mybir.SyncInfo(on_wait=waits[:1], on_update=[])
        for i in range(1, len(waits)):
            nxt = self.nc.sync.nop()
            nxt.ins.sync_info = mybir.SyncInfo(on_wait=waits[i : i + 1], on_update=[])
        self.nc.sync.drain()
        self.nc.all_engine_barrier()
        assert self.sems is not None
        popped = self.nc._tile_sem_poison_stack.pop()
        assert popped is self._sem_poison
        self.nc.clear_and_free_semaphores(list(self.sems.allocated().values()))
        self.nc.all_engine_barrier()

    def __exit__(self, exc_type, exc_val, exc_tb):
        r = super().__exit__(exc_type, exc_val, exc_tb)
        if exc_type is None:
            _split_multi_waits(self.nc)
        return r


# ---------------------------------------------------------------------------
# problem constants (hardcoded per the harness contract)
# ---------------------------------------------------------------------------
L = 8192          # tokens per sentence
EMB = 300
FD = 200          # F/G output dim
CORES = 8
SH = L // CORES   # per-core shard (1024)
SO = 304          # ones-augmented width (300 sen + 1 ones + 3 pad)
JBN = L // 128    # 64 j blocks
IBN = SH // 128   # 8 i blocks per shard

_nc_cache = {}


def _chunks(total, step):
    out = []
    o = 0
    while o < total:
        w = min(step, total - o)
        out.append((o, w))
        o += w
    return out


def _build(reps=1):
    """Build the SPMD per-core Bass program."""
    nc = bass.Bass()

    # ---- I/O ----
    s2tb = nc.dram_tensor("s2tb", [EMB, L], BF16, kind="ExternalInput")
    s1tsb = nc.dram_tensor("s1tsb", [EMB, SH], BF16, kind="ExternalInput")
    s1tsr = nc.dram_tensor("s1tsr", [EMB, SH], FP32R, kind="ExternalInput")
    s2tsr = nc.dram_tensor("s2tsr", [EMB, SH], FP32R, kind="ExternalInput")
    s2o = nc.dram_tensor("s2o", [L, SO], BF16, kind="ExternalInput")
    s1os = nc.dram_tensor("s1os", [SH, SO], BF16, kind="ExternalInput")
    fw1t = nc.dram_tensor("fw1t", [EMB, FD], BF16, kind="ExternalInput")
    fw2t = nc.dram_tensor("fw2t", [FD, FD], FP32R, kind="ExternalInput")
    gw1t = nc.dram_tensor("gw1t", [2 * EMB, FD], FP32R, kind="ExternalInput")
    gw2t = nc.dram_tensor("gw2t", [FD, FD], FP32R, kind="ExternalInput")
    gw1l = nc.dram_tensor("gw1l", [2 * EMB, FD], FP32R, kind="ExternalInput")
    gw2l = nc.dram_tensor("gw2l", [FD, FD], FP32R, kind="ExternalInput")
    fb1 = nc.dram_tensor("fb1", [FD, 1], F32, kind="ExternalInput")
    fb2 = nc.dram_tensor("fb2", [FD, 1], F32, kind="ExternalInput")
    gb1 = nc.dram_tensor("gb1", [FD, 1], F32, kind="ExternalInput")
    gb2 = nc.dram_tensor("gb2", [FD, 1], F32, kind="ExternalInput")
    ident = nc.dram_tensor("ident", [128, 128], F32, kind="ExternalInput")
    identb = nc.dram_tensor("identb", [128, 128], BF16, kind="ExternalInput")
    vsum = nc.dram_tensor("vsum", [4, 100], F32, kind="ExternalOutput")

    with _TC(nc) as tc:
        with (
            tc.tile_pool(name="persist", bufs=1) as pp,
            tc.tile_pool(name="work", bufs=2) as wp,
            tc.tile_pool(name="dram", bufs=1, space="DRAM") as dp,
        ):
            # ---- constants ----
            fw1_sb = [pp.tile([100, FD], BF16, tag=f"fw1_{i}", name=f"fw1_{i}") for i in range(3)]
            fw2_sb = [pp.tile([100, FD], FP32R, tag=f"fw2_{i}", name=f"fw2_{i}") for i in range(2)]
            gw1_sb = [pp.tile([100, FD], FP32R, tag=f"gw1_{i}", name=f"gw1_{i}") for i in range(6)]
            gw2_sb = [pp.tile([100, FD], FP32R, tag=f"gw2_{i}", name=f"gw2_{i}") for i in range(2)]
            gw1l_sb = [pp.tile([100, FD], FP32R, tag=f"gw1l_{i}", name=f"gw1l_{i}") for i in range(6)]
            gw2l_sb = [pp.tile([100, FD], FP32R, tag=f"gw2l_{i}", name=f"gw2l_{i}") for i in range(2)]
            for i, t in enumerate(fw1_sb):
                nc.sync.dma_start(t[:], fw1t[i * 100:(i + 1) * 100, :])
            for i, t in enumerate(fw2_sb):
                nc.sync.dma_start(t[:], fw2t[i * 100:(i + 1) * 100, :])
            biases = {}
            for nm, dr in (("fb1", fb1), ("fb2", fb2), ("gb1", gb1), ("gb2", gb2)):
                for h in range(2):
                    t = pp.tile([100, 1], F32, tag=f"{nm}_{h}", name=f"{nm}_{h}")
                    nc.sync.dma_start(t[:], dr[h * 100:(h + 1) * 100, :])
                    biases[(nm, h)] = t
            id_sb = pp.tile([128, 128], F32, tag="ident", name="id_sb")
            idb_sb = pp.tile([128, 128], BF16, tag="identb", name="idb_sb")

            # persistent activations
            FaTs = [pp.tile([100, SH], BF16, tag=f"FaTs{h}", name=f"FaTs{h}") for h in range(2)]
            NQ = L // 1024
            FbT = [[pp.tile([100, 1024], BF16, tag=f"FbT{h}_{q}", name=f"FbT{h}_{q}")
                    for q in range(NQ)] for h in range(2)]
            s1os_sb = [pp.tile([128, SO], BF16, tag=f"s1os_{k}", name=f"s1os_{k}")
                       for k in range(IBN)]
            so_sb = [pp.tile([128, SO], BF16, tag=f"so_{jb}", name=f"so_{jb}")
                     for jb in range(JBN)]
            # DRAM bounce for the per-i-half alpha-partial reduce-scatters
            n_in = [dp.tile([L, SO], BF16, name=f"n_in{h}") for h in range(2)]
            n_out = [dp.tile([SH, SO], BF16, name=f"n_out{h}") for h in range(2)]

            # ---------------- F MLP (feature-major, bf16 activations) -------
            def f_mlp(ps, src_dram, dst, chunk_order, post_chunk=None):
                for ci in chunk_order:
                    off = ci * 512
                    if post_chunk is not None:
                        post_chunk(ci)
                    xc = [wp.tile([100, 512], BF16, tag=f"f_x{c}", bufs=3, name=f"f_x{c}")
                          for c in range(3)]
                    for c in range(3):
                        nc.sync.dma_start(
                            xc[c][:], src_dram[c * 100:(c + 1) * 100, off:off + 512]
                        )
                    h1 = []
                    for m in range(2):
                        hp = ps.tile([100, 512], F32, tag=f"f_h1{m}", name=f"f_h1{m}")
                        for c in range(3):
                            nc.tensor.matmul(
                                hp[:], fw1_sb[c][:, m * 100:(m + 1) * 100],
                                xc[c][:], start=(c == 0), stop=(c == 2),
                            )
                        hs = wp.tile([100, 512], FP32R, tag=f"f_h1s{m}", bufs=3, name=f"f_h1s{m}")
                        nc.vector.tensor_scalar(
                            hs[:], hp[:], biases[("fb1", m)][:], 0.0,
                            mybir.AluOpType.add, mybir.AluOpType.max,
                        )
                        h1.append(hs)
                    for m in range(2):
                        hp = ps.tile([100, 512], F32, tag=f"f_h2{m}", name=f"f_h2{m}")
                        for c in range(2):
                            nc.tensor.matmul(
                                hp[:], fw2_sb[c][:, m * 100:(m + 1) * 100],
                                h1[c][:], start=(c == 0), stop=(c == 1),
                            )
                        nc.vector.tensor_scalar(
                            dst(m, off), hp[:], biases[("fb2", m)][:], 0.0,
                            mybir.AluOpType.add, mybir.AluOpType.max,
                        )

            # -------- G MLP + token sums --------
            def g_pass(x_dram, xT, row0):
                with tc.tile_pool(name="g_sb", bufs=2) as gp, \
                     tc.tile_pool(name="g_ps", bufs=2, space="PSUM") as ps:
                    nblk = _chunks(SH, 512)
                    vacc = [gp.tile([100, len(nblk)], F32, tag=f"vacc{m}", bufs=1,
                                    name=f"vacc{m}")
                            for m in range(2)]
                    for bi, (off, w) in enumerate(nblk):
                        xc = [gp.tile([100, 512], FP32R, tag=f"g_x{c}", name=f"g_x{c}")
                              for c in range(3)]
                        for c in range(3):
                            nc.sync.dma_start(
                                xc[c][:, :w],
                                x_dram[c * 100:(c + 1) * 100, off:off + w],
                            )
                        rhs6 = [t[:, :w] for t in xc] + \
                               [t[:, off:off + w] for t in xT]
                        h1 = []
                        for m in range(2):
                            hp = ps.tile([100, 512], F32, tag=f"g_h1{m}", name=f"g_h1{m}")
                            for c in range(12):
                                wsb = (gw1_sb if c < 6 else gw1l_sb)[c % 6]
                                nc.tensor.matmul(
                                    hp[:, :w], wsb[:, m * 100:(m + 1) * 100],
                                    rhs6[c % 6], start=(c == 0), stop=(c == 11),
                                )
                            hs = gp.tile([100, 512], FP32R, tag=f"g_h1s{m}", name=f"g_h1s{m}")
                            nc.vector.tensor_scalar(
                                hs[:, :w], hp[:, :w], biases[("gb1", m)][:], 0.0,
                                mybir.AluOpType.add, mybir.AluOpType.max,
                            )
                            h1.append(hs)
                        for m in range(2):
                            hp = ps.tile([100, 512], F32, tag=f"g_h2{m}", name=f"g_h2{m}")
                            for c in range(4):
                                wsb = (gw2_sb if c < 2 else gw2l_sb)[c % 2]
                                nc.tensor.matmul(
                                    hp[:, :w], wsb[:, m * 100:(m + 1) * 100],
                                    h1[c % 2][:, :w], start=(c == 0), stop=(c == 3),
                                )
                            vv = gp.tile([100, 512], F32, tag=f"g_v{m}", name=f"g_v{m}")
                            nc.scalar.activation(
                                vv[:, :w], hp[:, :w], AF.Relu,
                                bias=biases[("gb2", m)][:],
                                accum_out=vacc[m][:, bi:bi + 1],
                            )
                    for m in range(2):
                        tot = gp.tile([100, 1], F32, tag=f"tot{m}", bufs=1, name=f"tot{m}")
                        if len(nblk) == 1:
                            nc.vector.tensor_copy(tot[:], vacc[m][:, 0:1])
                        else:
                            nc.vector.tensor_add(
                                tot[:], vacc[m][:, 0:1], vacc[m][:, 1:2]
                            )
                            for bi in range(2, len(nblk)):
                                nc.vector.tensor_add(
                                    tot[:], tot[:], vacc[m][:, bi:bi + 1]
                                )
                        nc.sync.dma_start(vsum[row0 + m:row0 + m + 1, :], tot[:])

            # feature-major transpose of a [128, 300] f32 tile into outT tiles
            def emit_abT(ps, sp, src_f32, k, outT, tag="trf", bufs=2):
                for f in range(3):
                    trp = ps.tile([100, 128], F32, tag=tag, bufs=bufs, name="trf")
                    nc.tensor.transpose(
                        trp[:], src_f32[:, f * 100:(f + 1) * 100], id_sb[:]
                    )
                    nc.vector.tensor_copy(
                        outT[f][:, k * 128:(k + 1) * 128], trp[:]
                    )

            # ---------------- schedule ----------------
            for _rep in range(reps):
                if _rep == 0:
                    for i, t in enumerate(gw1_sb):
                        nc.sync.dma_start(t[:], gw1t[i * 100:(i + 1) * 100, :])
                    for i, t in enumerate(gw2_sb):
                        nc.sync.dma_start(t[:], gw2t[i * 100:(i + 1) * 100, :])
                    for i, t in enumerate(gw1l_sb):
                        nc.sync.dma_start(t[:], gw1l[i * 100:(i + 1) * 100, :])
                    for i, t in enumerate(gw2l_sb):
                        nc.sync.dma_start(t[:], gw2l[i * 100:(i + 1) * 100, :])
                    nc.sync.dma_start(id_sb[:], ident[:])
                    nc.sync.dma_start(idb_sb[:], identb[:])
                for k in range(IBN):
                    nc.sync.dma_start(
                        s1os_sb[k][:], s1os[k * 128:(k + 1) * 128, :]
                    )

                # F projections: Fa shard, then Fb full in jb consumption
                # order; s2o tile loads are interleaved between F chunks so
                # the E loop's beta matmuls aren't starved at sweep start.
                def so_loads(ci):
                    for jb in range(ci * 4, ci * 4 + 4):
                        nc.sync.dma_start(
                            so_sb[jb][:], s2o[jb * 128:(jb + 1) * 128, :]
                        )
                with tc.tile_pool(name="f_ps", bufs=2, space="PSUM") as ps1:
                    f_mlp(ps1, s1tsb,
                          lambda m, off: FaTs[m][:, off:off + 512], [0, 1])
                    f_mlp(ps1, s2tb,
                          lambda m, off: FbT[m][(off // 1024)][:, off % 1024:off % 1024 + 512],
                          list(range(16)), post_chunk=so_loads)

                # ---- fused E pass: beta accumulators + alpha partials ----
                betaT = [pp.tile([100, SH], FP32R, tag=f"betaT{f}", name=f"betaT{f}")
                         for f in range(3)]
                alphaT = [pp.tile([100, SH], FP32R, tag=f"alphaT{f}", name=f"alphaT{f}")
                          for f in range(3)]
                with tc.tile_pool(name="sp_sb", bufs=2) as sp, \
                     tc.tile_pool(name="sp_ps", bufs=1, space="PSUM") as ps:
                    for ih in range(2):
                        accs_b = [ps.tile([128, SO], F32, tag=f"accb{k}",
                                          name=f"accb{k}") for k in range(4)]
                        for jb in range(JBN):
                            qt, qo = jb // 8, (jb % 8) * 128
                            acc_a = ps.tile([128, SO], F32, tag="acca", bufs=1,
                                            name="acc_a")
                            e_ps = ps.tile([128, 512], F32, tag="e", bufs=2,
                                           name="e_ps")
                            for c in range(2):
                                nc.tensor.matmul(
                                    e_ps[:], FbT[c][qt][:, qo:qo + 128],
                                    FaTs[c][:, ih * 512:ih * 512 + 512],
                                    start=(c == 0), stop=(c == 1),
                                )
                            p_t = sp.tile([128, 512], BF16, tag="p", bufs=3,
                                          name="p_t")
                            nc.scalar.activation(p_t[:], e_ps[:], AF.Exp)
                            for k in range(4):
                                nc.tensor.matmul(
                                    accs_b[k][:],
                                    p_t[:, k * 128:(k + 1) * 128], so_sb[jb][:],
                                    start=(jb == 0), stop=(jb == JBN - 1),
                                )
                            pT = sp.tile([128, 512], BF16, tag="pT", bufs=2,
                                         name="pT")
                            trp = ps.tile([128, 512], BF16, tag="trp", bufs=1,
                                          name="trp")
                            for k in range(4):
                                nc.tensor.transpose(
                                    trp[:, k * 128:(k + 1) * 128],
                                    p_t[:, k * 128:(k + 1) * 128], idb_sb[:],
                                )
                            nc.vector.tensor_copy(pT[:], trp[:])
                            for k in range(4):
                                nc.tensor.matmul(
                                    acc_a[:], pT[:, k * 128:(k + 1) * 128],
                                    s1os_sb[ih * 4 + k][:],
                                    start=(k == 0), stop=(k == 3),
                                )
                            n_t = sp.tile([128, SO], BF16, tag="nt", bufs=2,
                                          name="n_t")
                            nc.scalar.activation(n_t[:], acc_a[:], AF.Copy)
                            nc.sync.dma_start(
                                n_in[ih][jb * 128:(jb + 1) * 128, :], n_t[:]
                            )
                        nc.gpsimd.collective_compute(
                            "ReduceScatter", mybir.AluOpType.add,
                            replica_groups=[list(range(CORES))],
                            ins=[n_in[ih][:, :].opt()],
                            outs=[n_out[ih][:, :].opt()],
                        )
                        # beta finalize for this i-half: divide + transpose
                        # (reuse the "e" PSUM slots for the transposes)
                        for k in range(4):
                            rec = sp.tile([128, 1], F32, tag="rec", bufs=2,
                                          name="rec")
                            nc.vector.reciprocal(rec[:], accs_b[k][:, 300:301])
                            dv = sp.tile([128, 300], F32, tag="dv", bufs=2,
                                         name="dv")
                            nc.vector.tensor_scalar_mul(
                                dv[:], accs_b[k][:, 0:300], rec[:]
                            )
                            emit_abT(ps, sp, dv, ih * 4 + k, betaT, tag="e", bufs=2)

                g_pass(s1tsr, betaT, 0)

                # alpha finalize: sum the two half reduce-scatters, divide,
                # transpose to feature-major
                with tc.tile_pool(name="af_sb", bufs=2) as asp, \
                     tc.tile_pool(name="af_ps", bufs=1, space="PSUM") as aps:
                    for k in range(IBN):
                        nsb = [asp.tile([128, SO], BF16, tag=f"nsb{h}", bufs=2,
                                        name=f"nsb{h}") for h in range(2)]
                        for h in range(2):
                            nc.sync.dma_start(
                                nsb[h][:], n_out[h][k * 128:(k + 1) * 128, :]
                            )
                        ns = asp.tile([128, SO], F32, tag="ns", bufs=2, name="ns")
                        nc.vector.tensor_add(ns[:], nsb[0][:], nsb[1][:])
                        rec = asp.tile([128, 1], F32, tag="rec", bufs=2, name="rec")
                        nc.vector.reciprocal(rec[:], ns[:, 300:301])
                        dv = asp.tile([128, 300], F32, tag="dv", bufs=2, name="dv")
                        nc.vector.tensor_scalar_mul(dv[:], ns[:, 0:300], rec[:])
                        emit_abT(aps, asp, dv, k, alphaT)

                g_pass(s2tsr, alphaT, 2)

    return nc


def _round_fp32r(a):
    ai = np.ascontiguousarray(a, dtype=np.float32).view(np.uint32)
    return ((ai + np.uint32(1 << 11)) & np.uint32(0xFFFFF000)).view(np.float32)


def _prep_common(sen1, sen2, F_w1, F_b1, F_w2, F_b2, G_w1, G_b1, G_w2, G_b2):
    import ml_dtypes
    sen1 = np.asarray(sen1, np.float32)
    sen2 = np.asarray(sen2, np.float32)

    def onesaug(s):
        o = np.zeros((s.shape[0], SO), np.float32)
        o[:, :EMB] = s
        o[:, EMB] = 1.0
        return o.astype(ml_dtypes.bfloat16)

    g1t = _round_fp32r(np.asarray(G_w1, np.float32).T)
    g2t = _round_fp32r(np.asarray(G_w2, np.float32).T)
    common = {
        "s2tb": sen2.T.astype(ml_dtypes.bfloat16),
        "s2o": onesaug(sen2),
        "fw1t": np.asarray(F_w1, np.float32).T.astype(ml_dtypes.bfloat16),
        "fw2t": _round_fp32r(np.asarray(F_w2, np.float32).T),
        "gw1t": g1t, "gw2t": g2t,
        "gw1l": _round_fp32r(np.asarray(G_w1, np.float32).T - g1t),
        "gw2l": _round_fp32r(np.asarray(G_w2, np.float32).T - g2t),
        "fb1": np.asarray(F_b1, np.float32).reshape(FD, 1),
        "fb2": np.asarray(F_b2, np.float32).reshape(FD, 1),
        "gb1": np.asarray(G_b1, np.float32).reshape(FD, 1),
        "gb2": np.asarray(G_b2, np.float32).reshape(FD, 1),
        "ident": np.eye(128, dtype=np.float32),
        "identb": np.eye(128, dtype=np.float32).astype(ml_dtypes.bfloat16),
    }
    s1t_r = _round_fp32r(sen1.T)
    s2t_r = _round_fp32r(sen2.T)
    s1t_b = sen1.T.astype(ml_dtypes.bfloat16)
    s1o_full = onesaug(sen1)
    per_core = []
    for c in range(CORES):
        m = dict(common)
        m["s1tsb"] = np.ascontiguousarray(s1t_b[:, c * SH:(c + 1) * SH])
        m["s1tsr"] = np.ascontiguousarray(s1t_r[:, c * SH:(c + 1) * SH])
        m["s2tsr"] = np.ascontiguousarray(s2t_r[:, c * SH:(c + 1) * SH])
        m["s1os"] = np.ascontiguousarray(s1o_full[c * SH:(c + 1) * SH, :])
        per_core.append(m)
    return per_core


def kernel(sen1, sen2, F_w1, F_b1, F_w2, F_b2,
           G_w1, G_b1, G_w2, G_b2, H_w1, H_b1, H_w2, H_b2):
    sen1 = np.asarray(sen1, np.float32)
    sen2 = np.asarray(sen2, np.float32)
    assert sen1.shape == (L, EMB) and sen2.shape == (L, EMB)

    if "nc" not in _nc_cache:
        _nc_cache["nc"] = _build()
    nc = _nc_cache["nc"]

    in_maps = _prep_common(sen1, sen2, F_w1, F_b1, F_w2, F_b2,
                           G_w1, G_b1, G_w2, G_b2)
    res = run_bass_kernel_spmd(nc, in_maps, list(range(CORES))).results

    hx = np.zeros(2 * FD, np.float64)
    for c in range(CORES):
        v = res[c]["vsum"].astype(np.float64)
        hx[0:100] += v[0]
        hx[100:200] += v[1]
        hx[200:300] += v[2]
        hx[300:400] += v[3]

    h = np.maximum(hx @ np.asarray(H_w1, np.float64).T + np.asarray(H_b1, np.float64), 0)
    logits = h @ np.asarray(H_w2, np.float64).T + np.asarray(H_b2, np.float64)
    p = np.exp(logits - logits.max())
    p = p / p.sum()
    return p.astype(np.float32)
